# revision 13
# baseline (speedup 1.0000x reference)
"""GAT (2-layer, PyG-style GATConv) on 8 Trainium2 NeuronCores.

v3 strategy (dst-sharded):
- Nodes sharded by dst across 8 cores (12500 each); edges partitioned by dst
  core; segment softmax + weighted aggregation local per dst shard.
- Table rows are fp16 [h(16) | a_s | pad] = 18 elems (36B) at 256B stride,
  gathered per-edge with the custom SWDGE dma_gather (4 queues).
- Source nodes are assigned to 4 index chunks by a greedy balancer that
  equalizes each dst node's in-edge count across chunks (padding ~34 -> ~17%).
- Consecutive dst tiles are grouped (<=6 tiles, ni <= 8192) so each gather
  instruction covers a whole group: ~72 gathers/launch instead of 392, which
  cuts the serial SWDGE descriptor-generation overhead on the Pool engine.
- Scores/softmax/reduction run as group-wide DVE/ACT passes; epilogues are
  batched per group. Self loops are handled locally (no gather slots).
- Both layers share one schedule; segment max is dropped (safe exp range).
- Transform runs k-block-major: 4 contiguous 6.4MB x-block DMAs, 98 PSUM
  -resident accumulators, one batched epilogue.
- 3 SPMD launches: transform / layer-1 aggregation / layer-2 aggregation.
"""

import contextlib

import numpy as np

import concourse.ap_utils as ap_utils
import concourse.bacc as bacc
import concourse.bass as bass
import concourse.mybir as mybir
from concourse.bass import round_up_to_multiple
from concourse.bass_utils import run_bass_kernel_spmd
from concourse.masks import make_identity
from concourse.tile import TileContext

P = 128
NCORES = 8
N = 100000
F_IN = 512
HID = 16
C_OUT = 32
NEG_SLOPE = 0.2
W = 18                 # fp16 row: h(16) | a_s | pad
ROWE = 128             # table row stride in fp16 elems (256B)
NCHUNK = 4
CAP = 32700            # max rows per chunk (int16 dummy must fit)
SH = N // NCORES
T_TILES = (SH + P - 1) // P
SHP = T_TILES * P
NEG_BIG = -60000.0     # fp16-representable; kills pad slots in softmax
MAXNI = 64             # max (G * K_cc) per gather -> ni <= 8192
GMAX = 6

FP = mybir.dt.float32
F16 = mybir.dt.float16
I16 = mybir.dt.int16


def _my_dma_gather(gp, out_ap, in_ap, idxs_ap, num_idxs, elem_size,
                   elem_step, queue_num):
    """BassGpSimd.dma_gather (non-transpose, DRAM source) without the
    256B-elem_size restriction; elem_step must still be a 256B multiple."""
    assert idxs_ap.dtype == I16
    assert in_ap.dtype == out_ap.dtype
    assert in_ap.space == bass.MemorySpace.DRAM
    assert idxs_ap.space == bass.MemorySpace.SBUF
    assert out_ap.space == bass.MemorySpace.SBUF
    assert ap_utils.ap_is_contiguous(out_ap.ap[1:])
    assert ap_utils.ap_is_contiguous(idxs_ap.ap[1:])
    assert in_ap.ap[-1][1] == out_ap.ap[-1][1] == elem_size
    assert out_ap.ap[0][1] * out_ap.ap[1][1] == round_up_to_multiple(num_idxs, 128)
    assert in_ap.ap[0][0] == elem_step
    stride_bytes = elem_step * mybir.dt.size(in_ap.dtype)
    assert stride_bytes % 256 == 0 and stride_bytes // 256 < 256
    _in_ap = gp.lower_ap_dma(in_ap, for_custom_bir_dma=True)
    _idxs_ap = gp.lower_ap(idxs_ap)
    _out_ap = gp.lower_ap(out_ap)
    return gp.add_instruction(
        mybir.InstDMAGatherAnt(
            name=gp.bass.get_next_instruction_name(),
            ins=[*_in_ap, _idxs_ap, gp.lower_val_access(gp.to_reg(num_idxs))],
            outs=[_out_ap],
            transpose=False,
            num_idxs=num_idxs,
            elem_size=elem_size,
            stride_bytes_256=stride_bytes // 256,
            gen_mode=0,
            single_packet=False,
            queue_num=queue_num,
        )
    )


# ---------------------------------------------------------------------------
# Host-side preprocessing
# ---------------------------------------------------------------------------

def _greedy_chunks(src, dst):
    """Assign each source node to one of 4 chunks, balancing every dst node's
    in-edge count across chunks (3 passes of greedy/refinement)."""
    order_e = np.argsort(src, kind="stable")
    src_s = src[order_e]
    dst_s = dst[order_e]
    starts = np.searchsorted(src_s, np.arange(N))
    ends = np.searchsorted(src_s, np.arange(N) + 1)
    outdeg = ends - starts

    cnt_nc = np.zeros((N, NCHUNK), np.int32)
    chunk_of = np.full(N, -1, np.int32)
    sizes = np.zeros(NCHUNK, np.int64)
    sorder = np.argsort(-outdeg, kind="stable")
    rng = np.random.default_rng(0)
    tie = rng.random((N, NCHUNK)) * 1e-3
    for s in sorder:
        ds = dst_s[starts[s]:ends[s]]
        scores = cnt_nc[ds, :].sum(axis=0).astype(np.float64)
        scores[sizes >= CAP] = np.inf
        cc = int(np.argmin(scores + tie[s]))
        chunk_of[s] = cc
        sizes[cc] += 1
        np.add.at(cnt_nc, (ds, cc), 1)
    for _ in range(2):
        for s in sorder:
            ds = dst_s[starts[s]:ends[s]]
            cur = chunk_of[s]
            cnt_nc[ds, cur] -= 1
            sizes[cur] -= 1
            scores = cnt_nc[ds, :].sum(axis=0).astype(np.float64)
            scores[sizes >= CAP] = np.inf
            cc = int(np.argmin(scores))
            chunk_of[s] = cc
            sizes[cc] += 1
            np.add.at(cnt_nc, (ds, cc), 1)
    return chunk_of, sizes


def _schedule(src, dst, chunk_of, sizes):
    """Shared group schedule for both layers.

    Grid layout per tile group (t0, G): chunk-major slabs; within chunk cc a
    [G * Kg_cc, 128] rectangle, tile g at columns [g*Kg_cc, (g+1)*Kg_cc);
    slot (col j, partition p) is gather index j*128 + p.
    """
    row0 = np.zeros(NCHUNK, np.int64)
    acc = 0
    for cc in range(NCHUNK):
        row0[cc] = acc
        acc += int(sizes[cc]) + 1
    ntab = acc

    local_idx = np.zeros(N, np.int64)
    for cc in range(NCHUNK):
        ids = np.where(chunk_of == cc)[0]
        local_idx[ids] = np.arange(len(ids))
    table_row_of = row0[chunk_of] + local_idx

    core_e = dst // SH
    ch_e = chunk_of[src]
    cnts, lists = [], []
    for c in range(NCORES):
        m = core_e == c
        s_c = src[m]
        d_loc = dst[m] - c * SH
        cnt = np.zeros((SH, NCHUNK), np.int32)
        np.add.at(cnt, (d_loc, ch_e[m]), 1)
        cnts.append(cnt)
        lists.append((s_c, d_loc, ch_e[m]))

    orders = []
    K = np.zeros((T_TILES, NCHUNK), np.int64)
    for c in range(NCORES):
        cnt = cnts[c]
        mx = cnt.max(axis=1)
        order = np.lexsort((-cnt[:, 1], -cnt[:, 0], -mx))
        orders.append(order)
        cs = np.concatenate(
            [cnt[order], np.zeros((SHP - SH, NCHUNK), np.int32)]
        ).reshape(T_TILES, P, NCHUNK)
        K = np.maximum(K, cs.max(axis=1))
    K = np.maximum(K, 1)

    # pack consecutive tiles into groups
    groups = []
    t = 0
    while t < T_TILES:
        g = 1
        while t + g < T_TILES and g < GMAX:
            cand = K[t:t + g + 1].max(axis=0)
            if int(((g + 1) * cand).max()) > MAXNI:
                break
            g += 1
        groups.append((t, g))
        t += g
    Kg = [K[t0:t0 + g].max(axis=0) for (t0, g) in groups]

    # slab offsets (in slot columns of 128)
    slab = np.zeros((len(groups), NCHUNK), np.int64)
    acc = 0
    for gi, (t0, g) in enumerate(groups):
        for cc in range(NCHUNK):
            slab[gi, cc] = acc
            acc += int(Kg[gi][cc]) * g
    total_cols = acc
    total = total_cols * P

    group_of_tile = np.zeros(T_TILES, np.int64)
    gin_of_tile = np.zeros(T_TILES, np.int64)
    for gi, (t0, g) in enumerate(groups):
        group_of_tile[t0:t0 + g] = gi
        gin_of_tile[t0:t0 + g] = np.arange(g)
    kg_arr = np.stack(Kg)  # [ngroups, NCHUNK]

    # per-slot dummy chunk index, shared across cores
    dummy = np.empty(total, np.int64)
    cum = 0
    for gi, (t0, g) in enumerate(groups):
        for cc in range(NCHUNK):
            n = int(Kg[gi][cc]) * g * P
            dummy[cum:cum + n] = sizes[cc]
            cum += n

    idx_list = []
    for c in range(NCORES):
        s_c, d_loc, ch = lists[c]
        order = orders[c]
        gridpos_of_node = np.empty(SH, np.int64)
        gridpos_of_node[order] = np.arange(SH)
        gp_e = gridpos_of_node[d_loc]
        t_e = gp_e // P
        p_e = gp_e % P
        gi_e = group_of_tile[t_e]
        gin_e = gin_of_tile[t_e]
        bucket = gp_e * NCHUNK + ch
        bo = np.argsort(bucket, kind="stable")
        bsort = bucket[bo]
        rank = np.arange(len(bsort)) - np.searchsorted(bsort, bsort, "left")
        rank_e = np.empty_like(rank)
        rank_e[bo] = rank
        col_e = slab[gi_e, ch] + gin_e * kg_arr[gi_e, ch] + rank_e
        slotpos = col_e * P + p_e
        arr = np.full(total, -1, np.int64)
        arr[slotpos] = local_idx[s_c]
        pad = arr < 0
        arr[pad] = dummy[pad]
        w = arr.reshape(total // 16, 16).T.astype(np.int16)
        idx_list.append(np.tile(w, (8, 1)))
    return (groups, Kg, idx_list, orders, table_row_of, row0, ntab, total)


# ---------------------------------------------------------------------------
# Device programs
# ---------------------------------------------------------------------------

def _build_transform(reps=1, calib=False):
    """Launch 1: h = x_shard @ W1 ; a_s = h@att_src ; a_d = h@att_dst.
    k-block-major: contiguous x-block DMAs, PSUM-resident per-tile accums.
    Outputs: tabs [SHP, 18] fp16 rows [h|a_s|0], ad [SHP, 1] fp32."""
    nc = bacc.Bacc("TRN2", target_bir_lowering=False, debug=False,
                   num_devices=NCORES)
    xkind = "Internal" if calib else "ExternalInput"
    xt = nc.dram_tensor("xt", [F_IN, SH], FP, kind=xkind).ap()
    w1 = nc.dram_tensor("w1", [F_IN // P, P, HID], FP, kind=xkind).ap()
    # attw: [P, 2*T_TILES*HID] = att_src replicated per tile | att_dst repl.
    attw = nc.dram_tensor("attw", [P, 2 * T_TILES * HID], FP, kind=xkind).ap()
    tabs = nc.dram_tensor("tabs", [SHP, W], F16, kind="ExternalOutput").ap()
    ad = nc.dram_tensor("ad", [SHP, 1], FP, kind="ExternalOutput").ap()
    KC = F_IN // P
    TH = T_TILES * HID
    with TileContext(nc) as tc:
        with tc.tile_pool(name="cst", bufs=1) as cst, \
             tc.tile_pool(name="xb", bufs=2) as xbp, \
             tc.tile_pool(name="ep", bufs=1) as epp, \
             tc.tile_pool(name="ps", bufs=4, space="PSUM") as ps:
            w1t = cst.tile([P, KC * HID], FP)
            nc.sync.dma_start(out=w1t[:].rearrange("p (k h) -> p k h", k=KC),
                              in_=w1[:].rearrange("k p h -> p k h"))
            attt = cst.tile([P, 2 * TH], FP)
            nc.sync.dma_start(out=attt[:], in_=attw[:])
            rep_cm = tc.For_i(0, reps) if reps > 1 else contextlib.nullcontext()
            with rep_cm:
                h32 = epp.tile([P, TH], FP, tag="h32")
                nc.vector.memset(h32[:], 0.0)
                TB = 25  # tiles per node block
                for b0 in range(0, T_TILES, TB):
                    nb = min(TB, T_TILES - b0)
                    c0 = b0 * P
                    cn = min(nb * P, SH - c0)
                    xbs = []
                    for k in range(KC):
                        xbk = xbp.tile([P, TB * P], FP, tag=f"xb{k}",
                                       name=f"xb{k}")
                        nc.sync.dma_start(
                            out=xbk[:, 0:cn],
                            in_=xt[k * P:(k + 1) * P, c0:c0 + cn])
                        xbs.append(xbk)
                    for tl in range(nb):
                        t = b0 + tl
                        m0 = tl * P
                        mn = min(P, cn - m0)
                        psum = ps.tile([P, HID], FP, space="PSUM", tag="psum")
                        for k in range(KC):
                            nc.tensor.matmul(
                                psum[:mn, :],
                                lhsT=xbs[k][:, m0:m0 + mn],
                                rhs=w1t[:, k * HID:(k + 1) * HID],
                                start=(k == 0), stop=(k == KC - 1))
                        nc.scalar.copy(h32[:mn, t * HID:(t + 1) * HID],
                                       psum[:mn, :])
                scr = epp.tile([P, TH], FP, tag="scr")
                as_all = epp.tile([P, T_TILES], FP, tag="asall")
                ad_all = epp.tile([P, T_TILES], FP, tag="adall")
                nc.vector.tensor_tensor(out=scr[:], in0=h32[:],
                                        in1=attt[:, 0:TH],
                                        op=mybir.AluOpType.mult)
                nc.vector.tensor_reduce(
                    as_all[:], scr[:].rearrange("p (t h) -> p t h", h=HID),
                    axis=mybir.AxisListType.X, op=mybir.AluOpType.add)
                nc.vector.tensor_tensor(out=scr[:], in0=h32[:],
                                        in1=attt[:, TH:2 * TH],
                                        op=mybir.AluOpType.mult)
                nc.vector.tensor_reduce(
                    ad_all[:], scr[:].rearrange("p (t h) -> p t h", h=HID),
                    axis=mybir.AxisListType.X, op=mybir.AluOpType.add)
                roww = epp.tile([P, T_TILES * W], F16, tag="roww")
                nc.vector.memset(roww[:], 0.0)
                nc.scalar.copy(
                    roww[:].rearrange("p (t w) -> p t w", w=W)[:, :, 0:HID],
                    h32[:].rearrange("p (t h) -> p t h", h=HID))
                nc.scalar.copy(
                    roww[:].rearrange("p (t w) -> p t w", w=W)[:, :, HID:HID + 1],
                    as_all[:].rearrange("p (t o) -> p t o", o=1))
                nc.sync.dma_start(
                    out=tabs[:].rearrange("(t p) w -> p t w", p=P),
                    in_=roww[:].rearrange("p (t w) -> p t w", w=W))
                nc.sync.dma_start(
                    out=ad[:].rearrange("(t p) o -> p t o", p=P),
                    in_=ad_all[:].rearrange("p (t o) -> p t o", o=1))
    nc.compile()
    return nc


def _build_aggregate(groups, Kg, ntab, layer, reps=1, calib_idx=None,
                     total=None):
    """Launches 2 & 3: grouped grid gather + segment softmax + aggregation.

    layer == 1: h' = relu(num/den + b1); emits [h'|a_s2|0] fp16 + ad2.
    layer == 2: out = log_softmax((num/den) @ W2 + b2).
    """
    nc = bacc.Bacc("TRN2", target_bir_lowering=False, debug=False,
                   num_devices=NCORES, num_swdge_queues=4)
    TOTW = total // 16
    ikind = "Internal" if calib_idx is not None else "ExternalInput"
    tab = nc.dram_tensor("tab", [ntab, ROWE], F16, kind=ikind).ap()
    if calib_idx is not None:
        idx = nc.inline_tensor(calib_idx, name="idx").ap()
    else:
        idx = nc.dram_tensor("idx", [P, TOTW], I16, kind="ExternalInput").ap()
    adg = nc.dram_tensor("adg", [SHP, 1], FP, kind=ikind).ap()
    own = nc.dram_tensor("own", [SHP, W], F16, kind=ikind).ap()
    if layer == 1:
        # vecs: b1 | u2 | v2, each replicated GMAX times: [P, 3*GMAX*HID]
        vecs = nc.dram_tensor("vecs", [P, 3 * GMAX * HID], FP, kind=ikind).ap()
        tabs = nc.dram_tensor("tabs", [SHP, W], F16, kind="ExternalOutput").ap()
        ad = nc.dram_tensor("ad", [SHP, 1], FP, kind="ExternalOutput").ap()
    else:
        # vecs: b2 replicated GMAX times: [P, GMAX*C_OUT]
        vecs = nc.dram_tensor("vecs", [P, GMAX * C_OUT], FP, kind=ikind).ap()
        w2 = nc.dram_tensor("w2", [HID, C_OUT], FP, kind=ikind).ap()
        y = nc.dram_tensor("y", [SHP, C_OUT], FP, kind="ExternalOutput").ap()

    with TileContext(nc) as tc:
        with tc.tile_pool(name="cst", bufs=1) as cst, \
             tc.tile_pool(name="ix", bufs=3) as ixp, \
             tc.tile_pool(name="gr", bufs=2) as grp, \
             tc.tile_pool(name="gw", bufs=2) as gwp, \
             tc.tile_pool(name="sc", bufs=2) as scp, \
             tc.tile_pool(name="ou", bufs=2) as oup, \
             tc.tile_pool(name="ps", bufs=2, space="PSUM") as ps:
            vt = cst.tile([P, vecs.shape[1]], FP)
            nc.sync.dma_start(out=vt[:], in_=vecs[:])
            if layer == 2:
                w2t = cst.tile([HID, C_OUT], FP)
                nc.sync.dma_start(out=w2t[:], in_=w2[:])
                ident = cst.tile([P, P], FP)
                make_identity(nc, ident[:])
            rep_cm = tc.For_i(0, reps) if reps > 1 else contextlib.nullcontext()
            with rep_cm:
                col_off = 0
                for gi, (t0, G) in enumerate(groups):
                    kg = [int(Kg[gi][cc]) for cc in range(NCHUNK)]
                    S = sum(kg) * G          # slot columns in this group
                    r0, r1 = t0 * P, (t0 + G) * P
                    g = grp.tile([P, S * W], F16, tag="grid")
                    idx_t = ixp.tile([P, S * 8], I16, tag="idx")
                    nc.sync.dma_start(
                        out=idx_t[:], in_=idx[:, col_off * 8:(col_off + S) * 8])
                    coff = 0
                    for cc in range(NCHUNK):
                        ni = kg[cc] * G * P
                        _my_dma_gather(
                            nc.gpsimd,
                            g[:, coff * W:(coff + kg[cc] * G) * W].rearrange(
                                "p (k w) -> p k w", w=W),
                            tab[ROW0[cc]:, 0:W],
                            idx_t[:, coff * 8:(coff + kg[cc] * G) * 8],
                            ni, W, ROWE, cc)
                        coff += kg[cc] * G
                    adw = scp.tile([P, G], FP, tag="adw")
                    nc.sync.dma_start(
                        out=adw[:].rearrange("p (g o) -> p g o", o=1),
                        in_=adg[r0:r1, :].rearrange("(g p) o -> p g o", p=P))
                    hown = oup.tile([P, G * W], F16, tag="hown")
                    nc.sync.dma_start(
                        out=hown[:].rearrange("p (g w) -> p g w", w=W),
                        in_=own[r0:r1, :].rearrange("(g p) w -> p g w", p=P))
                    # pre = a_s + a_d  (per chunk slab: bcast adw over k)
                    pre = scp.tile([P, S], FP, tag="pre")
                    coff = 0
                    for cc in range(NCHUNK):
                        kc = kg[cc]
                        nc.vector.tensor_tensor(
                            out=pre[:, coff:coff + kc * G].rearrange(
                                "p (g k) -> p g k", k=kc),
                            in0=g[:, coff * W:(coff + kc * G) * W].rearrange(
                                "p (k w) -> p k w", w=W)[:, :, HID:HID + 1]
                            .rearrange("p k w -> p (k w)").rearrange(
                                "p (g k) -> p g k", k=kc),
                            in1=adw[:].to_broadcast([P, G, kc]),
                            op=mybir.AluOpType.add)
                        coff += kc * G
                    wts = scp.tile([P, S], FP, tag="wts")
                    nc.scalar.activation(wts[:], pre[:],
                                         mybir.ActivationFunctionType.Lrelu,
                                         bias=0.0, scale=1.0, alpha=NEG_SLOPE)
                    nc.scalar.activation(wts[:], wts[:],
                                         mybir.ActivationFunctionType.Exp)
                    # den per tile: reduce each chunk slab, then across chunks
                    den4 = scp.tile([P, NCHUNK * G], FP, tag="den4")
                    coff = 0
                    for cc in range(NCHUNK):
                        kc = kg[cc]
                        nc.vector.tensor_reduce(
                            den4[:, cc * G:(cc + 1) * G],
                            wts[:, coff:coff + kc * G].rearrange(
                                "p (g k) -> p g k", k=kc),
                            axis=mybir.AxisListType.X, op=mybir.AluOpType.add)
                        coff += kc * G
                    den = scp.tile([P, G], FP, tag="den")
                    nc.vector.tensor_reduce(
                        den[:], den4[:].rearrange("p (c g) -> p g c", g=G),
                        axis=mybir.AxisListType.X, op=mybir.AluOpType.add)
                    # self-loop score
                    pres = scp.tile([P, G], FP, tag="pres")
                    nc.vector.tensor_tensor(
                        out=pres[:],
                        in0=hown[:].rearrange("p (g w) -> p g w", w=W)
                        [:, :, HID:HID + 1].rearrange("p g w -> p (g w)"),
                        in1=adw[:], op=mybir.AluOpType.add)
                    ws = scp.tile([P, G], FP, tag="ws")
                    nc.scalar.activation(ws[:], pres[:],
                                         mybir.ActivationFunctionType.Lrelu,
                                         bias=0.0, scale=1.0, alpha=NEG_SLOPE)
                    nc.scalar.activation(ws[:], ws[:],
                                         mybir.ActivationFunctionType.Exp)
                    nc.vector.tensor_tensor(out=den[:], in0=den[:], in1=ws[:],
                                            op=mybir.AluOpType.add)
                    inv = scp.tile([P, G], FP, tag="inv")
                    nc.vector.reciprocal(inv[:], den[:])
                    nc.vector.tensor_tensor(out=ws[:], in0=ws[:], in1=inv[:],
                                            op=mybir.AluOpType.mult)
                    # alpha = wts * inv (per chunk slab bcast)
                    coff = 0
                    for cc in range(NCHUNK):
                        kc = kg[cc]
                        nc.vector.tensor_tensor(
                            out=wts[:, coff:coff + kc * G].rearrange(
                                "p (g k) -> p g k", k=kc),
                            in0=wts[:, coff:coff + kc * G].rearrange(
                                "p (g k) -> p g k", k=kc),
                            in1=inv[:].to_broadcast([P, G, kc]),
                            op=mybir.AluOpType.mult)
                        coff += kc * G
                    # weighted grid (fp32) and per-(tile,chunk) reduction
                    gwt = gwp.tile([P, S * W], FP, tag="gwt")
                    nc.vector.tensor_tensor(
                        out=gwt[:].rearrange("p (k w) -> p k w", w=W),
                        in0=g[:].rearrange("p (k w) -> p k w", w=W),
                        in1=wts[:].to_broadcast([P, S, W]),
                        op=mybir.AluOpType.mult)
                    num4 = oup.tile([P, NCHUNK * G * W], FP, tag="num4")
                    coff = 0
                    for cc in range(NCHUNK):
                        kc = kg[cc]
                        nc.vector.tensor_reduce(
                            num4[:, cc * G * W:(cc + 1) * G * W].rearrange(
                                "p (g w) -> p g w", w=W),
                            gwt[:, coff * W:(coff + kc * G) * W].rearrange(
                                "p (g k w) -> p g w k", k=kc, w=W),
                            axis=mybir.AxisListType.X, op=mybir.AluOpType.add)
                        coff += kc * G
                    num = oup.tile([P, G * W], FP, tag="num")
                    nc.vector.tensor_reduce(
                        num[:], num4[:].rearrange("p (c gw) -> p gw c",
                                                  gw=G * W),
                        axis=mybir.AxisListType.X, op=mybir.AluOpType.add)
                    # + self loop: num += hown * (ws*inv)
                    nself = oup.tile([P, G * W], FP, tag="nself")
                    nc.vector.tensor_tensor(
                        out=nself[:].rearrange("p (g w) -> p g w", w=W),
                        in0=hown[:].rearrange("p (g w) -> p g w", w=W),
                        in1=ws[:].to_broadcast([P, G, W]),
                        op=mybir.AluOpType.mult)
                    nc.vector.tensor_tensor(out=num[:], in0=num[:],
                                            in1=nself[:],
                                            op=mybir.AluOpType.add)
                    nv = num[:].rearrange("p (g w) -> p g w", w=W)
                    if layer == 1:
                        hp32 = oup.tile([P, G * HID], FP, tag="hp32")
                        nc.vector.tensor_tensor(
                            out=hp32[:].rearrange("p (g h) -> p g h", h=HID),
                            in0=nv[:, :, 0:HID],
                            in1=vt[:, 0:G * HID].rearrange(
                                "p (g h) -> p g h", h=HID),
                            op=mybir.AluOpType.add)
                        nc.vector.tensor_scalar_max(hp32[:], hp32[:], 0.0)
                        scr = oup.tile([P, G * HID], FP, tag="scr")
                        as2 = oup.tile([P, G], FP, tag="as2")
                        ad2 = oup.tile([P, G], FP, tag="ad2")
                        nc.vector.tensor_tensor(
                            out=scr[:], in0=hp32[:],
                            in1=vt[:, GMAX * HID:(GMAX + G) * HID],
                            op=mybir.AluOpType.mult)
                        nc.vector.tensor_reduce(
                            as2[:], scr[:].rearrange("p (g h) -> p g h", h=HID),
                            axis=mybir.AxisListType.X, op=mybir.AluOpType.add)
                        nc.vector.tensor_tensor(
                            out=scr[:], in0=hp32[:],
                            in1=vt[:, 2 * GMAX * HID:(2 * GMAX + G) * HID],
                            op=mybir.AluOpType.mult)
                        nc.vector.tensor_reduce(
                            ad2[:], scr[:].rearrange("p (g h) -> p g h", h=HID),
                            axis=mybir.AxisListType.X, op=mybir.AluOpType.add)
                        roww = oup.tile([P, G * W], F16, tag="roww")
                        nc.vector.memset(roww[:], 0.0)
                        nc.scalar.copy(
                            roww[:].rearrange("p (g w) -> p g w", w=W)
                            [:, :, 0:HID],
                            hp32[:].rearrange("p (g h) -> p g h", h=HID))
                        nc.scalar.copy(
                            roww[:].rearrange("p (g w) -> p g w", w=W)
                            [:, :, HID:HID + 1],
                            as2[:].rearrange("p (g o) -> p g o", o=1))
                        nc.sync.dma_start(
                            out=tabs[r0:r1, :].rearrange("(g p) w -> p g w",
                                                         p=P),
                            in_=roww[:].rearrange("p (g w) -> p g w", w=W))
                        nc.sync.dma_start(
                            out=ad[r0:r1, :].rearrange("(g p) o -> p g o", p=P),
                            in_=ad2[:].rearrange("p (g o) -> p g o", o=1))
                    else:
                        logw = oup.tile([P, G * C_OUT], FP, tag="logw")
                        for gg in range(G):
                            pT = ps.tile([HID, P], FP, space="PSUM", tag="pT")
                            nc.tensor.transpose(pT[:], nv[:, gg, 0:HID],
                                                ident[:])
                            nT = oup.tile([HID, P], FP, tag="nT")
                            nc.scalar.copy(nT[:], pT[:])
                            p2 = ps.tile([P, C_OUT], FP, space="PSUM", tag="p2")
                            nc.tensor.matmul(p2[:], lhsT=nT[:], rhs=w2t[:],
                                             start=True, stop=True)
                            nc.scalar.copy(
                                logw[:, gg * C_OUT:(gg + 1) * C_OUT], p2[:])
                        nc.vector.tensor_tensor(
                            out=logw[:], in0=logw[:], in1=vt[:, 0:G * C_OUT],
                            op=mybir.AluOpType.add)
                        mx = scp.tile([P, G], FP, tag="mx")
                        nc.vector.tensor_reduce(
                            mx[:], logw[:].rearrange("p (g c) -> p g c",
                                                     c=C_OUT),
                            axis=mybir.AxisListType.X,
                            op=mybir.AluOpType.max, negate=True)
                        exw = oup.tile([P, G * C_OUT], FP, tag="exw")
                        nc.vector.tensor_tensor(
                            out=exw[:].rearrange("p (g c) -> p g c", c=C_OUT),
                            in0=logw[:].rearrange("p (g c) -> p g c", c=C_OUT),
                            in1=mx[:].to_broadcast([P, G, C_OUT]),
                            op=mybir.AluOpType.add)
                        nc.scalar.activation(exw[:], exw[:],
                                             mybir.ActivationFunctionType.Exp)
                        se = scp.tile([P, G], FP, tag="se")
                        nc.vector.tensor_reduce(
                            se[:], exw[:].rearrange("p (g c) -> p g c",
                                                    c=C_OUT),
                            axis=mybir.AxisListType.X, op=mybir.AluOpType.add)
                        ls = scp.tile([P, G], FP, tag="ls")
                        nc.scalar.activation(ls[:], se[:],
                                             mybir.ActivationFunctionType.Ln)
                        ofs = scp.tile([P, G], FP, tag="ofs")
                        nc.vector.tensor_tensor(out=ofs[:], in0=mx[:],
                                                in1=ls[:],
                                                op=mybir.AluOpType.subtract)
                        nc.vector.tensor_tensor(
                            out=logw[:].rearrange("p (g c) -> p g c", c=C_OUT),
                            in0=logw[:].rearrange("p (g c) -> p g c", c=C_OUT),
                            in1=ofs[:].to_broadcast([P, G, C_OUT]),
                            op=mybir.AluOpType.add)
                        nc.sync.dma_start(
                            out=y[r0:r1, :].rearrange("(g p) c -> p g c", p=P),
                            in_=logw[:].rearrange("p (g c) -> p g c", c=C_OUT))
                    col_off += S
    nc.compile()
    return nc


ROW0 = None  # set by kernel() before building aggregate programs

# ---------------------------------------------------------------------------
# Main entry
# ---------------------------------------------------------------------------

LAST_TIMINGS = {}
LAST_STATS = {}


def _run_retry(nc, in_maps, cores):
    try:
        return run_bass_kernel_spmd(nc, in_maps, cores)
    except Exception:
        return run_bass_kernel_spmd(nc, in_maps, cores)


def kernel(x, edge_index, W1, att_src1, att_dst1, b1, W2, att_src2, att_dst2, b2):
    global ROW0
    import time as _time
    x = np.asarray(x, np.float32)
    W1 = np.asarray(W1, np.float32)
    W2 = np.asarray(W2, np.float32)
    att_src1 = np.asarray(att_src1, np.float32)
    att_dst1 = np.asarray(att_dst1, np.float32)
    att_src2 = np.asarray(att_src2, np.float32)
    att_dst2 = np.asarray(att_dst2, np.float32)
    b1 = np.asarray(b1, np.float32)
    b2 = np.asarray(b2, np.float32)
    src = np.asarray(edge_index[0], dtype=np.int64)
    dst = np.asarray(edge_index[1], dtype=np.int64)

    print("preprocess...", flush=True)
    _t = _time.time()
    chunk_of, sizes = _greedy_chunks(src, dst)
    (groups, Kg, idx_list, orders, table_row_of, row0, ntab,
     total) = _schedule(src, dst, chunk_of, sizes)
    ROW0 = row0
    LAST_TIMINGS["preprocess"] = _time.time() - _t
    LAST_STATS["slots1"] = total
    LAST_STATS["slots2"] = total
    print(f"slots/layer/core: {total} (pad "
          f"{(total*NCORES-len(src))/len(src)*100:.1f}%), groups "
          f"{len(groups)}, preprocess {LAST_TIMINGS['preprocess']:.1f}s",
          flush=True)

    # ---- launch 1: transform -------------------------------------------
    print("build1...", flush=True)
    nc1 = _build_transform()
    xT = np.ascontiguousarray(x.T)
    attw = np.zeros((P, 2 * T_TILES * HID), np.float32)
    attw[:, 0:T_TILES * HID] = np.tile(att_src1, T_TILES)[None, :]
    attw[:, T_TILES * HID:] = np.tile(att_dst1, T_TILES)[None, :]
    w1r = np.ascontiguousarray(W1.reshape(F_IN // P, P, HID))
    in1 = [{"xt": np.ascontiguousarray(xT[:, c * SH:(c + 1) * SH]),
            "w1": w1r, "attw": attw}
           for c in range(NCORES)]
    _t = _time.time()
    r1 = _run_retry(nc1, in1, list(range(NCORES)))
    LAST_TIMINGS["launch1"] = _time.time() - _t
    print("launch1 done", flush=True)

    tab_rows = np.zeros((ntab, ROWE), np.float16)
    ad1 = np.zeros(N, np.float32)
    for c in range(NCORES):
        ids = c * SH + np.arange(SH)
        tab_rows[table_row_of[ids], 0:W] = r1.results[c]["tabs"][:SH, :]
        ad1[ids] = r1.results[c]["ad"][:SH, 0]
    for cc in range(NCHUNK):
        tab_rows[row0[cc] + sizes[cc], HID] = NEG_BIG  # dummy a_src

    # ---- launch 2: layer-1 aggregation ---------------------------------
    print("build2...", flush=True)
    nc2 = _build_aggregate(groups, Kg, ntab, layer=1, total=total)
    u2 = W2 @ att_src2
    v2 = W2 @ att_dst2
    vecs1 = np.zeros((P, 3 * GMAX * HID), np.float32)
    vecs1[:, 0:GMAX * HID] = np.tile(b1, GMAX)[None, :]
    vecs1[:, GMAX * HID:2 * GMAX * HID] = np.tile(u2, GMAX)[None, :]
    vecs1[:, 2 * GMAX * HID:] = np.tile(v2, GMAX)[None, :]
    in2 = []
    for c in range(NCORES):
        ids_sorted = c * SH + orders[c]
        adg = np.zeros((SHP, 1), np.float32)
        adg[:SH, 0] = ad1[ids_sorted]
        ownr = np.zeros((SHP, W), np.float16)
        ownr[:SH] = tab_rows[table_row_of[ids_sorted], 0:W]
        in2.append({"tab": tab_rows, "idx": idx_list[c], "adg": adg,
                    "own": ownr, "vecs": vecs1})
    _t = _time.time()
    r2 = _run_retry(nc2, in2, list(range(NCORES)))
    LAST_TIMINGS["launch2"] = _time.time() - _t
    print("launch2 done", flush=True)

    tab2 = np.zeros((ntab, ROWE), np.float16)
    ad2 = np.zeros(N, np.float32)
    for c in range(NCORES):
        ids_sorted = c * SH + orders[c]
        tab2[table_row_of[ids_sorted], 0:W] = r2.results[c]["tabs"][:SH, :]
        ad2[ids_sorted] = r2.results[c]["ad"][:SH, 0]
    for cc in range(NCHUNK):
        tab2[row0[cc] + sizes[cc], HID] = NEG_BIG

    # ---- launch 3: layer-2 aggregation + classifier --------------------
    print("build3...", flush=True)
    nc3 = _build_aggregate(groups, Kg, ntab, layer=2, total=total)
    vecs2 = np.zeros((P, GMAX * C_OUT), np.float32)
    vecs2[:, :] = np.tile(b2, GMAX)[None, :]
    in3 = []
    for c in range(NCORES):
        ids_sorted = c * SH + orders[c]
        adg = np.zeros((SHP, 1), np.float32)
        adg[:SH, 0] = ad2[ids_sorted]
        ownr = np.zeros((SHP, W), np.float16)
        ownr[:SH] = tab2[table_row_of[ids_sorted], 0:W]
        in3.append({"tab": tab2, "idx": idx_list[c], "adg": adg,
                    "own": ownr, "vecs": vecs2, "w2": W2})
    _t = _time.time()
    r3 = _run_retry(nc3, in3, list(range(NCORES)))
    LAST_TIMINGS["launch3"] = _time.time() - _t
    print("launch3 done", flush=True)

    out = np.zeros((N, C_OUT), np.float32)
    for c in range(NCORES):
        out[c * SH + orders[c]] = r3.results[c]["y"][:SH, :]
    return out
